# revision 27
# baseline (speedup 1.0000x reference)
"""GQA attention (B=2, S=2048, DIM=4096, H=32, KV=8, HD=128) with interleaved
RoPE + causal mask, distributed over 8 TRN2 NeuronCores.

Sharding: tensor-parallel over KV groups. Core c owns kv-group c (4 query
heads + 1 kv head): it projects Q/K/V for its group over ALL tokens (both
batches), runs causal attention locally, then an AllToAll (split in two
head-pair halves, bf16, overlapped with compute) re-shards the attention
output from head-major to token-major, and each core computes the output
projection for its 512-token slice against the full wo.

v2 vs baseline:
- everything the PE touches is bf16 (weights, x, q/k/v, probs, wo): bf16
  stationaries enable fast-weight-load, so LDWEIGHTS no longer serializes
  with the matmuls (fp32r stationaries cost 218ns LDW per 262ns matmul);
  DMA traffic halves. Accumulation stays fp32 in PSUM.
- projection weights loaded into SBUF once (baseline re-read them 8x).
- attention: q-blocks of 512; softmax denominator accumulated on the PE
  in PSUM via a ones-column matmul per k-tile (replaces 512 DVE adds);
  causal-diagonal k-tiles computed at partial width; k-tile loop runs the
  diagonal first (reversed) so the PSUM accumulation begins with a
  bank-clearing matmul and ends full-width.
- AllToAll split into two bf16 collectives (heads 0-1 / heads 2-3); the
  first overlaps the second attention pass, the second overlaps the start
  of the output projection (which accumulates A-half strips first).
"""
import sys
sys.path.insert(0, "/opt/trn_rl_repo")
import numpy as np

B, S, DIM = 2, 2048, 4096
H, KV, HD = 32, 8, 128
SCALE = HD ** -0.5
NC = 8
NT = B * S            # 4096 flat tokens
TC = 512              # token chunk for projections
QB = 512              # q block in attention
ND = DIM // 128       # 32 d-tiles

_CACHE = {}


def _build():
    import concourse.bacc as bacc
    import concourse.tile as tile
    from concourse import mybir

    F32 = mybir.dt.float32
    F32R = mybir.dt.float32r
    BF16 = mybir.dt.bfloat16
    EXP = mybir.ActivationFunctionType.Exp

    nc = bacc.Bacc("TRN2", target_bir_lowering=False, num_devices=NC)

    def param(name, shape, dt=F32):
        return nc.declare_dram_parameter(name, shape, dt, isOutput=False)

    xt = param("xt", [DIM, NT], BF16)      # x flattened+transposed, bf16
    wq_s = param("wq_s", [DIM, 512], BF16)  # my 4 heads' wq columns
    wk_s = param("wk_s", [DIM, 128], BF16)
    wv_s = param("wv_s", [DIM, 128], BF16)
    wo = param("wo", [DIM, DIM], BF16)
    cq = param("cq", [128, NT])            # RoPE tables, partition-pair layout
    sq = param("sq", [128, NT])            # (cq/sq scaled by 1/sqrt(HD))
    ck = param("ck", [128, NT])
    sk = param("sk", [128, NT])
    mask4 = param("mask4", [4, 128, 512], BF16)  # transposed causal diag blocks
    pswap = param("pswap", [128, 128], BF16)  # pair-swap permutation
    ident = param("ident", [128, 128], BF16)  # identity (PE transpose)
    ones = param("ones", [128, 128], BF16)
    out_ext = nc.declare_dram_parameter("out", [512, DIM], F32, isOutput=True)

    with tile.TileContext(nc) as tc:
        import contextlib
        with contextlib.ExitStack() as ctx:
            dram = ctx.enter_context(tc.tile_pool(name="dram", bufs=1, space="DRAM"))
            # two collectives: head pair 0-1 and head pair 2-3; rows 256/257
            # carry the (unnormalized) softmax denominators of the two heads
            # so normalization happens on the destination side, batched.
            a2a_inA = dram.tile([NC, 258, 512], BF16, name="a2a_inA")
            a2a_outA = dram.tile([NC, 258, 512], BF16, name="a2a_outA")
            a2a_inB = dram.tile([NC, 258, 512], BF16, name="a2a_inB")
            a2a_outB = dram.tile([NC, 258, 512], BF16, name="a2a_outB")

            consts = ctx.enter_context(tc.tile_pool(name="consts", bufs=1))
            pswap_sb = consts.tile([128, 128], BF16)
            ident_sb = consts.tile([128, 128], BF16)
            ones_bf = consts.tile([128, 128], BF16)
            mask_sb = [consts.tile([128, 512], BF16, name=f"mask{j}")
                       for j in range(4)]
            nc.sync.dma_start(out=pswap_sb, in_=pswap[:, :])
            nc.sync.dma_start(out=ident_sb, in_=ident[:, :])
            nc.sync.dma_start(out=ones_bf, in_=ones[:, :])
            for j in range(4):
                nc.sync.dma_start(out=mask_sb[j], in_=mask4[j, :, :])

            # out-proj input strips live across phases 2-4 (allocated before
            # the phase-1/2 pools so those can be released in stack order).
            otp = ctx.enter_context(tc.tile_pool(name="otp", bufs=32))
            f_order = []
            for p in range(NC):
                f_order += [(p, 0, a2a_outA), (p, 1, a2a_outA)]
            for p in range(NC):
                f_order += [(p, 2, a2a_outB), (p, 3, a2a_outB)]
            ot_sb = [otp.tile([128, 512], BF16, tag="ot", name=f"ot{i}")
                     for i in range(32)]
            dnp = ctx.enter_context(tc.tile_pool(name="dnp", bufs=2))
            dn_all = [dnp.tile([16, 512], BF16, tag="dna", name=f"dn_{ab}")
                      for ab in "AB"]
            rec_all = [dnp.tile([16, 512], F32, tag="reca", name=f"rec_{ab}")
                       for ab in "AB"]
            nrm = ctx.enter_context(tc.tile_pool(name="nrm", bufs=2))
            # wo/y pools live outside the phase-1/2 pool stack so the wo
            # prefetch DMAs are not fenced behind the qt/wgt SBUF release
            wop = ctx.enter_context(tc.tile_pool(name="wop", bufs=6))
            ysbp = ctx.enter_context(tc.tile_pool(name="ysb", bufs=4))

            # persistent per-core tensors
            ph12 = contextlib.ExitStack()
            qt_pool = ph12.enter_context(tc.tile_pool(name="qt", bufs=4))
            ktv_pool = ph12.enter_context(tc.tile_pool(name="ktv", bufs=1))
            qt_sb = [qt_pool.tile([128, NT], BF16, tag="qt", name=f"qt{i}")
                     for i in range(4)]
            kt_sb = ktv_pool.tile([128, NT], BF16, tag="ktv", name="kt_sb")
            v_sb = ktv_pool.tile([128, NT], BF16, tag="vtv", name="v_sb")

            # projection weights: resident in SBUF for all of phase 1
            wgt = ph12.enter_context(tc.tile_pool(name="wgt", bufs=1))
            wq_sb = wgt.tile([128, ND * 512], BF16, name="wq_sb")
            wk_sb = wgt.tile([128, ND * 128], BF16, name="wk_sb")
            wv_sb = wgt.tile([128, ND * 128], BF16, name="wv_sb")
            for d in range(ND):
                ds_ = slice(d * 128, (d + 1) * 128)
                nc.scalar.dma_start(out=wq_sb[:, d * 512:(d + 1) * 512],
                                    in_=wq_s[ds_, :])
                nc.scalar.dma_start(out=wk_sb[:, d * 128:(d + 1) * 128],
                                    in_=wk_s[ds_, :])
                nc.scalar.dma_start(out=wv_sb[:, d * 128:(d + 1) * 128],
                                    in_=wv_s[ds_, :])

            # ---------------- Phase 1: QKV projection + RoPE ----------------
            with (
                tc.tile_pool(name="xw", bufs=4) as xw,
                tc.tile_pool(name="cs", bufs=2) as csp,
                tc.tile_pool(name="rope", bufs=4) as rope,
                tc.tile_pool(name="pj_ps", bufs=4, space="PSUM") as pj_ps,
                tc.tile_pool(name="kv_ps", bufs=2, space="PSUM") as kv_ps,
                tc.tile_pool(name="sw_ps", bufs=2, space="PSUM") as sw_ps,
            ):
                for tcb in range(NT // TC):
                    ts = slice(tcb * TC, (tcb + 1) * TC)
                    qps = [pj_ps.tile([128, TC], F32, tag="qps", name=f"qps{i}")
                           for i in range(4)]
                    kps = kv_ps.tile([128, TC], F32, tag="kvps")
                    vps = kv_ps.tile([128, TC], F32, tag="kvps")
                    for d in range(ND):
                        xt_t = xw.tile([128, TC], BF16, tag="xt_t")
                        nc.sync.dma_start(out=xt_t,
                                          in_=xt[d * 128:(d + 1) * 128, ts])
                        st = (d == 0)
                        sp = (d == ND - 1)
                        nc.tensor.matmul(kps[:, :],
                                         wk_sb[:, d * 128:(d + 1) * 128],
                                         xt_t[:, :], start=st, stop=sp)
                        nc.tensor.matmul(vps[:, :],
                                         wv_sb[:, d * 128:(d + 1) * 128],
                                         xt_t[:, :], start=st, stop=sp)
                        for f in range(4):
                            nc.tensor.matmul(
                                qps[f][:, :],
                                wq_sb[:, d * 512 + f * 128:d * 512 + (f + 1) * 128],
                                xt_t[:, :], start=st, stop=sp)

                    cq_t = csp.tile([128, TC], F32, tag="cq_t")
                    sq_t = csp.tile([128, TC], F32, tag="sq_t")
                    ck_t = csp.tile([128, TC], F32, tag="ck_t")
                    sk_t = csp.tile([128, TC], F32, tag="sk_t")
                    nc.scalar.dma_start(out=cq_t, in_=cq[:, ts])
                    nc.scalar.dma_start(out=sq_t, in_=sq[:, ts])
                    nc.scalar.dma_start(out=ck_t, in_=ck[:, ts])
                    nc.scalar.dma_start(out=sk_t, in_=sk[:, ts])

                    # RoPE(t) = t*C + (P@t)*S ; K/V first so their DVE chain
                    # overlaps the Q matmuls of the next token chunk.
                    kraw = rope.tile([128, TC], BF16, tag="raw")
                    nc.vector.tensor_copy(kraw[:, :], kps[:, :])
                    kswp = sw_ps.tile([128, TC], F32, tag="swp")
                    nc.tensor.matmul(kswp[:, :], pswap_sb[:, :], kraw[:, :],
                                     start=True, stop=True)
                    t1 = rope.tile([128, TC], F32, tag="t1")
                    nc.vector.tensor_mul(t1[:, :], kraw[:, :], ck_t[:, :])
                    t2 = rope.tile([128, TC], F32, tag="t2")
                    nc.vector.tensor_mul(t2[:, :], kswp[:, :], sk_t[:, :])
                    nc.vector.tensor_add(kt_sb[:, ts], t1[:, :], t2[:, :])

                    # V: evacuate then PE-transpose to token-major tiles
                    vraw = rope.tile([128, TC], BF16, tag="vraw")
                    nc.vector.tensor_copy(vraw[:, :], vps[:, :])
                    for tt in range(4):
                        tps = sw_ps.tile([128, 128], BF16, tag="swp", name="tps")
                        nc.tensor.transpose(tps[:, :],
                                            vraw[:, tt * 128:(tt + 1) * 128],
                                            ident_sb[:, :])
                        gtt = tcb * 4 + tt
                        nc.vector.tensor_copy(v_sb[:, gtt * 128:(gtt + 1) * 128],
                                              tps[:, :])

                    for f in range(4):
                        raw = rope.tile([128, TC], BF16, tag="raw")
                        nc.vector.tensor_copy(raw[:, :], qps[f][:, :])
                        swp = sw_ps.tile([128, TC], F32, tag="swp")
                        nc.tensor.matmul(swp[:, :], pswap_sb[:, :], raw[:, :],
                                         start=True, stop=True)
                        t1 = rope.tile([128, TC], F32, tag="t1")
                        nc.vector.tensor_mul(t1[:, :], raw[:, :], cq_t[:, :])
                        t2 = rope.tile([128, TC], F32, tag="t2")
                        nc.vector.tensor_mul(t2[:, :], swp[:, :], sq_t[:, :])
                        nc.vector.tensor_add(qt_sb[f][:, ts], t1[:, :], t2[:, :])

            # ---------------- Phase 2: causal attention (local group) -------
            # two passes over (b, qb): heads 0-1 then heads 2-3, each pass
            # feeding its own AllToAll so the collectives overlap compute.
            with (
                tc.tile_pool(name="att", bufs=8) as att,
                tc.tile_pool(name="osb", bufs=4) as osbp,
                tc.tile_pool(name="sp_ps", bufs=3, space="PSUM") as sp_ps,
                tc.tile_pool(name="op_ps", bufs=3, space="PSUM") as op_ps,
                tc.tile_pool(name="dn_ps", bufs=2, space="PSUM") as dn_ps,
            ):
                def attn_pass(h0, a2a_in):
                    for b in range(B):
                        for qb in range(4):
                            dest = b * 4 + qb
                            q0 = b * S + qb * QB
                            n_k = 4 * (qb + 1)
                            ops = [op_ps.tile([128, QB], F32, tag="ops",
                                              name=f"ops{i}") for i in range(2)]
                            dns = [dn_ps.tile([1, QB], F32, tag="dn",
                                              name=f"dn{i}") for i in range(2)]
                            # k-tile order: full tiles first (bank-clearing
                            # start), diagonal tiles last ending full-width;
                            # late diagonals keep their DVE mask-adds clear
                            # of the previous group's evacuation copies.
                            kts = list(range(n_k - 4)) \
                                + [n_k - 1, n_k - 2, n_k - 3, n_k - 4]
                            for i, kt in enumerate(kts):
                                k0 = b * S + kt * 128
                                j = kt - (n_k - 4)  # >=0: diagonal tile index
                                w0 = 128 * j if j > 0 else 0  # valid q-col start
                                st = (i == 0)
                                sp = (i == n_k - 1)
                                ktile = kt_sb[:, k0:k0 + 128]
                                vtile = v_sb[:, k0:k0 + 128]
                                ets = []
                                for hh in range(2):
                                    h = h0 + hh
                                    sps = sp_ps.tile([128, QB], F32, tag="sps")
                                    nc.tensor.matmul(
                                        sps[:, w0:QB], ktile,
                                        qt_sb[h][:, q0 + w0:q0 + QB],
                                        start=True, stop=True)
                                    et = att.tile([128, QB], BF16, tag="et")
                                    if j >= 0:
                                        nc.vector.tensor_add(
                                            sps[:, w0:w0 + 128],
                                            sps[:, w0:w0 + 128],
                                            mask_sb[j][:, w0:w0 + 128])
                                    nc.scalar.activation(et[:, w0:QB],
                                                         sps[:, w0:QB], EXP)
                                    ets.append(et)
                                for hh in range(2):
                                    nc.tensor.matmul(
                                        ops[hh][:, w0:QB], vtile,
                                        ets[hh][:, w0:QB],
                                        start=st, stop=sp)
                                for hh in range(2):
                                    nc.tensor.matmul(
                                        dns[hh][0:1, w0:QB], ones_bf[:, 0:1],
                                        ets[hh][:, w0:QB],
                                        start=st, stop=sp)
                            # stage unnormalized AV + denominators into the
                            # collective buffer (normalization is dest-side)
                            for hh in range(2):
                                osb = osbp.tile([128, QB], BF16, tag="osb")
                                nc.vector.tensor_copy(osb[:, :], ops[hh][:, :])
                                nc.scalar.dma_start(
                                    out=a2a_in[dest, hh * 128:(hh + 1) * 128, :],
                                    in_=osb[:, :])
                                dsb = osbp.tile([1, QB], BF16, tag="dsb")
                                nc.vector.tensor_copy(dsb[:, :], dns[hh][:, :])
                                nc.scalar.dma_start(
                                    out=a2a_in[dest, 256 + hh:257 + hh, :],
                                    in_=dsb[:, :])

                def load_and_normalize(half, ab):
                    # pull denominator rows first (they gate the batched
                    # reciprocal), then strips; scale each strip by its
                    # per-token 1/denominator.
                    i0 = 16 * half
                    for i, (p, hh, buf) in enumerate(f_order[i0:i0 + 16],
                                                     start=i0):
                        nc.gpsimd.dma_start(
                            out=dn_all[ab][i - i0:i - i0 + 1, :],
                            in_=buf[p, 256 + hh % 2:257 + hh % 2, :])
                    for i, (p, hh, buf) in enumerate(f_order[i0:i0 + 16],
                                                     start=i0):
                        r0 = (hh % 2) * 128
                        nc.gpsimd.dma_start(out=ot_sb[i],
                                            in_=buf[p, r0:r0 + 128, :])
                    nc.vector.reciprocal(rec_all[ab][:, :], dn_all[ab][:, :])
                    for i in range(i0, i0 + 16):
                        stag = nrm.tile([1, 512], F32, tag="stag")
                        nc.gpsimd.dma_start(
                            out=stag[:, :],
                            in_=rec_all[ab][i - i0:i - i0 + 1, :])
                        bsb = nrm.tile([128, 512], F32, tag="bsb")
                        nc.gpsimd.partition_broadcast(bsb[:, :], stag[:, :])
                        nc.vector.tensor_mul(ot_sb[i][:, :], ot_sb[i][:, :],
                                             bsb[:, :])

                attn_pass(0, a2a_inA)
                nc.gpsimd.collective_compute(
                    "AllToAll", mybir.AluOpType.bypass,
                    replica_groups=[list(range(NC))],
                    ins=[a2a_inA.opt()], outs=[a2a_outA.opt()],
                )
                attn_pass(2, a2a_inB)
                load_and_normalize(0, 0)
                nc.gpsimd.collective_compute(
                    "AllToAll", mybir.AluOpType.bypass,
                    replica_groups=[list(range(NC))],
                    ins=[a2a_inB.opt()], outs=[a2a_outB.opt()],
                )
                load_and_normalize(1, 1)

            ph12.close()  # release qt/ktv/wgt SBUF before out-proj pools

            # ---------------- Phase 4: output projection --------------------
            # ot strip (p, hh) = global feature rows 128*(4p+hh) of the 4096,
            # from a2a_outA (heads 0-1) / a2a_outB (heads 2-3). Accumulate
            # A-half strips first so the first dchunks start before the
            # second collective lands.
            with (
                tc.tile_pool(name="y_ps", bufs=8, space="PSUM") as y_ps,
            ):
                yps_of = {}

                def dchunk_mms(dchunk, lo, hi):
                    dsl = slice(dchunk * 512, (dchunk + 1) * 512)
                    if dchunk not in yps_of:
                        yps_of[dchunk] = [
                            y_ps.tile([128, 512], F32, tag="yps",
                                      name=f"yps{i}") for i in range(4)]
                    yps = yps_of[dchunk]
                    for i in range(lo, hi):
                        p, hh, buf = f_order[i]
                        f = p * 4 + hh  # global feature strip index
                        wo_t = wop.tile([128, 512], BF16, tag="wo_t")
                        nc.sync.dma_start(
                            out=wo_t, in_=wo[f * 128:(f + 1) * 128, dsl])
                        for tt in range(4):
                            nc.tensor.matmul(yps[tt][:, :],
                                             ot_sb[i][:, tt * 128:(tt + 1) * 128],
                                             wo_t[:, :],
                                             start=(i == 0), stop=(i == 31))

                def dchunk_evac(dchunk):
                    dsl = slice(dchunk * 512, (dchunk + 1) * 512)
                    for tt in range(4):
                        y_sb = ysbp.tile([128, 512], F32, tag="y_sb")
                        nc.vector.tensor_copy(y_sb[:, :],
                                              yps_of[dchunk][tt][:, :])
                        nc.scalar.dma_start(
                            out=out_ext[tt * 128:(tt + 1) * 128, dsl],
                            in_=y_sb[:, :])
                    del yps_of[dchunk]

                # dchunks 0/1: run both A-halves first so the PE has work
                # while the second collective + B-strip normalization land.
                dchunk_mms(0, 0, 16)
                dchunk_mms(1, 0, 16)
                dchunk_mms(0, 16, 32)
                dchunk_evac(0)
                dchunk_mms(1, 16, 32)
                dchunk_evac(1)
                for dchunk in range(2, 8):
                    dchunk_mms(dchunk, 0, 32)
                    dchunk_evac(dchunk)
    nc.compile()
    return nc


def _host_prep(x, freqs_cos, freqs_sin):
    from ml_dtypes import bfloat16
    xt = np.ascontiguousarray(x.reshape(NT, DIM).T.astype(bfloat16))
    pos = np.arange(NT) % S

    def cs(scale):
        c = np.empty((128, NT), np.float32)
        s = np.empty((128, NT), np.float32)
        ct, st_ = freqs_cos[pos].T * scale, freqs_sin[pos].T
        c[0::2] = ct
        c[1::2] = ct
        s[0::2] = -st_ * scale
        s[1::2] = st_ * scale
        return np.ascontiguousarray(c), np.ascontiguousarray(s)

    cq_, sq_ = cs(np.float32(SCALE))
    ck_, sk_ = cs(np.float32(1.0))
    pswap = np.zeros((128, 128), bfloat16)
    for i in range(128):
        pswap[i, i ^ 1] = 1.0
    # mask4[j]: [k=128, q=512] additive causal mask for the diagonal k-tile
    # whose k rows cover global q0+128j..q0+128j+127; only columns
    # [128j, 128j+128) are ever read (the triangular boundary window):
    # score(k, q) allowed iff q >= k.
    neg = np.float32(np.finfo(np.float32).min)
    mask4 = np.zeros((4, 128, 512), np.float32)  # cast below
    for j in range(4):
        for k in range(128):
            qq = j * 128 + np.arange(128)
            mask4[j, k, j * 128:(j + 1) * 128] = np.where(qq >= j * 128 + k,
                                                          0.0, neg)
    mask4 = mask4.astype(bfloat16)
    ident = np.eye(128, dtype=bfloat16)
    ones = np.ones((128, 128), bfloat16)
    return xt, cq_, sq_, ck_, sk_, pswap, mask4, ident, ones


def kernel(x, wq, wk, wv, wo, freqs_cos, freqs_sin, mask, positions):
    from concourse.bass_utils import run_bass_kernel_spmd
    from ml_dtypes import bfloat16

    x = np.asarray(x, np.float32)
    wq = np.asarray(wq, np.float32)
    wk = np.asarray(wk, np.float32)
    wv = np.asarray(wv, np.float32)
    wo = np.asarray(wo, np.float32)
    freqs_cos = np.asarray(freqs_cos, np.float32)
    freqs_sin = np.asarray(freqs_sin, np.float32)

    if "nc" not in _CACHE:
        _CACHE["nc"] = _build()
    nc = _CACHE["nc"]

    xt, cq_, sq_, ck_, sk_, pswap, mask4, ident, ones = _host_prep(
        x, freqs_cos, freqs_sin)
    wo_bf = np.ascontiguousarray(wo.astype(bfloat16))

    in_maps = []
    for c in range(NC):
        in_maps.append({
            "xt": xt,
            "wq_s": np.ascontiguousarray(
                wq[:, c * 512:(c + 1) * 512].astype(bfloat16)),
            "wk_s": np.ascontiguousarray(
                wk[:, c * 128:(c + 1) * 128].astype(bfloat16)),
            "wv_s": np.ascontiguousarray(
                wv[:, c * 128:(c + 1) * 128].astype(bfloat16)),
            "wo": wo_bf,
            "cq": cq_, "sq": sq_, "ck": ck_, "sk": sk_,
            "mask4": mask4, "pswap": pswap, "ident": ident, "ones": ones,
        })

    _CACHE["in_maps"] = in_maps
    res = run_bass_kernel_spmd(nc, in_maps, core_ids=list(range(NC)))
    out = np.empty((NT, DIM), np.float32)
    for c in range(NC):
        out[c * 512:(c + 1) * 512, :] = res.results[c]["out"]
    return out.reshape(B, S, DIM)
